# revision 12
# baseline (speedup 1.0000x reference)
"""AutoRec scoring kernel for 8x Trainium2 NeuronCores (Bass/Tile).

Computation (see problem reference):
    agg   = segment_sum(dat[:,None] * v[cols], rows, m)    # COO spmm (m,d)
    h     = sigmoid(agg + mu)                              # (m,d)
    score = sum(h[i] * w[j], -1) + b[j]                    # (P,)

Sharding: edges sharded by row range (8 equal ranges of 6250 rows), pairs
sharded by i range (same ranges) -> no cross-core communication at all.
v/w/mu/b replicated per core.

Per core:
  phase 1: for each 128-row block, gather v[col] rows (dma_gather, bf16),
           scatter-add edges to rows with a one-hot-times-dat matmul into
           PSUM (lhsT A[e,r] = dat(e) if row(e)==r), add mu via a K=1
           ones-matmul, sigmoid (ACT) -> h block -> DRAM.
  phase 2: gather w[j] and h[i_local] rows (dma_gather, bf16), fused
           multiply+reduce (tensor_tensor_reduce) -> per-pair scores.

Host does index preprocessing only (sort/bucket/pad + building the
block-one-hot dat matrices); all FLOPs and all gathers run on device.
"""

import os
import sys

import numpy as np

for _p in ("/opt/trn_rl_repo",):
    if os.path.isdir(_p) and _p not in sys.path:
        sys.path.insert(0, _p)

import ml_dtypes  # noqa: E402

import concourse.bacc as bacc  # noqa: E402
import concourse.bass as bass  # noqa: E402
import concourse.mybir as mybir  # noqa: E402
import concourse.tile as tile  # noqa: E402
from concourse.bass_utils import run_bass_kernel_spmd  # noqa: E402
from concourse.tile_rust import add_dep_helper  # noqa: E402

BF16 = ml_dtypes.bfloat16

NCORES = 8
M = 50000
N = 50000
D = 256
RPC = M // NCORES          # rows per core: 6250
BLOCKS = (RPC + 127) // 128  # 49
RPAD = BLOCKS * 128        # 6272
HALF = 32768               # int16 index limit; v/w split into two halves
G = 4096                   # phase-2 gather call size (slots)

_BUILD_CACHE = {}

# set by run (for test harness introspection)
LAST_RESULTS = None


def _build_program(CAPL, CAPH, LCAP, HCAP, WW, hdep_ranges):
    """Build the SPMD bass program. All cores run the same instructions;
    per-core behaviour differs only through input data.

    hdep_ranges: per phase-2 h-gather call, (min_block, max_block) of h rows
    it may touch (union over cores) -> explicit RAW deps onto the h writes.
    """
    CAP = CAPL + CAPH
    TS = LCAP + HCAP           # total phase-2 slots per core
    TT = TS // 128             # score tiles
    f32 = mybir.dt.float32
    bf16 = mybir.dt.bfloat16
    i16 = mybir.dt.int16

    nc = bacc.Bacc("TRN2", target_bir_lowering=False, debug=False,
                   num_devices=NCORES)

    # ---- DRAM tensors ----
    v_bf = nc.dram_tensor("v_bf", [N, D], bf16, kind="ExternalInput")
    a_t = nc.dram_tensor("a_t", [BLOCKS, 128, CAP * 128], bf16,
                         kind="ExternalInput")
    gi_lo = nc.dram_tensor("gi_lo", [BLOCKS, 128, CAPL * 8], i16,
                           kind="ExternalInput")
    gi_hi = nc.dram_tensor("gi_hi", [BLOCKS, 128, CAPH * 8], i16,
                           kind="ExternalInput")
    mu_bf = nc.dram_tensor("mu_bf", [1, D], bf16, kind="ExternalInput")
    w_bf = nc.dram_tensor("w_bf", [N, WW], bf16, kind="ExternalInput")
    wj_t = nc.dram_tensor("wj_t", [128, TS // 16], i16, kind="ExternalInput")
    hi_t = nc.dram_tensor("hi_t", [128, TS // 16], i16, kind="ExternalInput")
    h_dram = nc.dram_tensor("h_dram", [RPAD, WW], bf16)
    scores = nc.dram_tensor("scores", [128, TT], f32, kind="ExternalOutput")

    h_write_insts = [None] * BLOCKS

    with tile.TileContext(nc) as tc:
        import contextlib
        with contextlib.ExitStack() as ctx:
            const_p = ctx.enter_context(tc.tile_pool(name="const", bufs=1))
            a_p = ctx.enter_context(tc.tile_pool(name="a", bufs=2))
            gi_p = ctx.enter_context(tc.tile_pool(name="gi", bufs=2))
            x_p = ctx.enter_context(tc.tile_pool(name="x", bufs=2))
            h_p = ctx.enter_context(tc.tile_pool(name="h", bufs=2))
            ps_p = ctx.enter_context(
                tc.tile_pool(name="ps", bufs=2, space="PSUM"))
            wt_p = ctx.enter_context(tc.tile_pool(name="wt", bufs=2))
            ht_p = ctx.enter_context(tc.tile_pool(name="ht", bufs=2))
            pi_p = ctx.enter_context(tc.tile_pool(name="pi", bufs=2))
            pr_p = ctx.enter_context(tc.tile_pool(name="pr", bufs=2))

            ones_k1 = const_p.tile([1, 128], bf16)
            nc.vector.memset(ones_k1[:], 1.0)
            mu_sb = const_p.tile([1, D], bf16)
            nc.sync.dma_start(mu_sb[:], mu_bf[:, :])
            sc_sb = const_p.tile([128, TT], f32)

            # bisection flags (debug only)
            EN_P1 = os.environ.get("K_P1", "1") == "1"
            EN_P2 = os.environ.get("K_P2", "1") == "1"
            EN_HG = os.environ.get("K_HG", "1") == "1"
            EN_WG = os.environ.get("K_WG", "1") == "1"
            EN_MM = os.environ.get("K_MM", "1") == "1"

            # ---------------- phase 1 ----------------
            for blk in range(BLOCKS if EN_P1 else 0):
                at = a_p.tile([128, CAP * 128], bf16)
                nc.sync.dma_start(at[:], a_t[blk, :, :])
                gl = gi_p.tile([128, CAPL * 8], i16, tag="gil")
                nc.sync.dma_start(gl[:], gi_lo[blk, :, :])
                gh = gi_p.tile([128, CAPH * 8], i16, tag="gih")
                nc.sync.dma_start(gh[:], gi_hi[blk, :, :])

                xt = x_p.tile([128, CAP, D], bf16)
                nc.gpsimd.dma_gather(
                    xt[:, 0:CAPL, :], v_bf[0:HALF, :], gl[:],
                    num_idxs=CAPL * 128, num_idxs_reg=CAPL * 128,
                    elem_size=D, single_packet=False)
                nc.gpsimd.dma_gather(
                    xt[:, CAPL:CAP, :], v_bf[HALF:N, :], gh[:],
                    num_idxs=CAPH * 128, num_idxs_reg=CAPH * 128,
                    elem_size=D, single_packet=False)

                ht = h_p.tile([128, WW], bf16)
                if EN_MM:
                    ps = ps_p.tile([128, D], f32)
                    for c in range(CAP):
                        nc.tensor.matmul(
                            ps[:], lhsT=at[:, c * 128:(c + 1) * 128],
                            rhs=xt[:, c, :], start=(c == 0), stop=False)
                    nc.tensor.matmul(ps[:], lhsT=ones_k1[:], rhs=mu_sb[:],
                                     start=False, stop=True)
                    nc.scalar.activation(
                        ht[:, 0:D], ps[:],
                        mybir.ActivationFunctionType.Sigmoid)
                else:
                    nc.vector.memset(ht[:, 0:D], 0.5)
                if WW > D:
                    nc.vector.memset(ht[:, D:WW], 0.0)
                    nc.vector.memset(ht[:, D:D + 1], 1.0)
                wi = nc.sync.dma_start(
                    h_dram[blk * 128:(blk + 1) * 128, :], ht[:])
                h_write_insts[blk] = wi.ins

            # ---------------- phase 2 ----------------
            # w gathers: lo segment [0, LCAP) from w[0:HALF],
            #            hi segment [LCAP, TS) from w[HALF:]
            calls = []
            for off in range(0, LCAP, G):
                calls.append((off, min(G, LCAP - off), 0))
            for off in range(LCAP, TS, G):
                calls.append((off, min(G, LCAP + HCAP - off), 1))

            if not EN_P2:
                calls = []
                nc.vector.memset(sc_sb[:], 0.0)
            for ci, (off, sl, hseg) in enumerate(calls):
                nt = sl // 128
                wit = pi_p.tile([128, sl // 16], i16, tag="wit")
                nc.sync.dma_start(wit[:], wj_t[:, off // 16:(off + sl) // 16])
                hit = pi_p.tile([128, sl // 16], i16, tag="hit")
                nc.sync.dma_start(hit[:], hi_t[:, off // 16:(off + sl) // 16])

                wtile = wt_p.tile([128, nt, WW], bf16)
                if EN_WG:
                    wsrc = w_bf[0:HALF, :] if hseg == 0 else w_bf[HALF:N, :]
                    nc.gpsimd.dma_gather(
                        wtile[:], wsrc, wit[:], num_idxs=sl, num_idxs_reg=sl,
                        elem_size=WW, single_packet=False)
                else:
                    nc.vector.memset(wtile[:], 1.0)

                htile = ht_p.tile([128, nt, WW], bf16)
                if EN_HG:
                    gi_inst = nc.gpsimd.dma_gather(
                        htile[:], h_dram[:, :], hit[:], num_idxs=sl,
                        num_idxs_reg=sl, elem_size=WW, single_packet=False)
                    if EN_P1:
                        b0, b1 = hdep_ranges[ci]
                        for bb in range(b0, b1 + 1):
                            add_dep_helper(gi_inst.ins, h_write_insts[bb],
                                           reason="h RAW")
                else:
                    nc.vector.memset(htile[:], 1.0)

                pr = pr_p.tile([128, nt, WW], bf16)
                nc.vector.tensor_tensor(out=pr[:], in0=wtile[:],
                                        in1=htile[:],
                                        op=mybir.AluOpType.mult)
                c0 = off // 128
                nc.vector.tensor_reduce(
                    out=sc_sb[:, c0:c0 + nt], in_=pr[:],
                    axis=mybir.AxisListType.X, op=mybir.AluOpType.add)

            nc.sync.dma_start(scores[:, :], sc_sb[:])

    nc.compile()
    return nc


def kernel(idx, dat, m, n, i, j, v, mu, w, b):
    global LAST_RESULTS
    idx = np.asarray(idx)
    dat = np.asarray(dat, np.float32)
    i = np.asarray(i).astype(np.int64)
    j = np.asarray(j).astype(np.int64)
    v = np.asarray(v, np.float32)
    mu_np = np.asarray(mu, np.float32).reshape(1, D)
    w_np = np.asarray(w, np.float32)
    b_np = np.asarray(b, np.float32).reshape(-1)
    rows = idx[0].astype(np.int64)
    cols = idx[1].astype(np.int64)
    NNZ = rows.shape[0]
    P = i.shape[0]
    assert int(m) == M and int(n) == N
    assert v.shape == (N, D) and w_np.shape == (N, D)

    use_b = bool(np.any(b_np))
    WW = 384 if use_b else 256

    # ---------------- phase 1 host prep ----------------
    core_e = rows // RPC
    lrow = rows - core_e * RPC
    blk = lrow >> 7
    r_in_blk = (lrow & 127).astype(np.int64)
    half = (cols >= HALF).astype(np.int64)

    gkey = (core_e * BLOCKS + blk) * 2 + half
    order = np.argsort(gkey, kind="stable")
    gsorted = gkey[order]
    ngroups = NCORES * BLOCKS * 2
    counts = np.bincount(gsorted, minlength=ngroups)
    cnt_lo = counts[0::2]
    cnt_hi = counts[1::2]
    CAPL = max(1, int(np.ceil(cnt_lo.max() / 128)))
    CAPH = max(1, int(np.ceil(cnt_hi.max() / 128)))
    CAP = CAPL + CAPH
    gstart = np.zeros(ngroups + 1, np.int64)
    gstart[1:] = np.cumsum(counts)
    pos_in_group = np.arange(NNZ) - gstart[gsorted]
    eslot = pos_in_group + (gsorted % 2) * (CAPL * 128)
    g2 = gsorted // 2
    e_core = g2 // BLOCKS
    e_blk = g2 % BLOCKS
    echunk = eslot >> 7
    e_in_chunk = eslot & 127

    # A[core, blk, e, chunk, r] = dat ; lhsT layout per chunk is [e, r]
    A = np.zeros((NCORES, BLOCKS, 128, CAP, 128), BF16)
    A[e_core, e_blk, e_in_chunk, echunk, r_in_blk[order]] = \
        dat[order].astype(BF16)

    gi = np.zeros((NCORES, BLOCKS, CAP * 128), np.int16)
    colv = (cols[order] - half[order] * HALF).astype(np.int16)
    gi[e_core, e_blk, eslot] = colv
    # idx tiles: 16-partition interleave (pos k -> [k%16, k//16]),
    # replicated 8x across partition groups (one copy per Q7 core)
    gil = gi[:, :, :CAPL * 128].reshape(NCORES, BLOCKS, CAPL * 8, 16)
    gih = gi[:, :, CAPL * 128:].reshape(NCORES, BLOCKS, CAPH * 8, 16)
    gi_lo = np.tile(gil.swapaxes(2, 3), (1, 1, 8, 1))
    gi_hi = np.tile(gih.swapaxes(2, 3), (1, 1, 8, 1))

    # ---------------- phase 2 host prep ----------------
    p_core = i // RPC
    il = (i - p_core * RPC).astype(np.int64)
    jhalf = (j >= HALF).astype(np.int64)
    porder = np.lexsort((il, jhalf, p_core))
    key2 = (p_core * 2 + jhalf)[porder]
    pcounts = np.bincount(key2, minlength=NCORES * 2)
    plo = pcounts[0::2]
    phi = pcounts[1::2]
    LCAP = int(np.ceil(max(1, plo.max()) / 128)) * 128
    HCAP = int(np.ceil(max(1, phi.max()) / 128)) * 128
    TS = LCAP + HCAP
    TT = TS // 128

    pstart = np.zeros(NCORES * 2 + 1, np.int64)
    pstart[1:] = np.cumsum(pcounts)
    pos2 = np.arange(P) - pstart[key2]
    slot = pos2 + (key2 % 2) * LCAP
    pcs = key2 // 2
    wj = np.zeros((NCORES, TS), np.int16)
    hi_ = np.zeros((NCORES, TS), np.int16)
    wj[pcs, slot] = (j[porder] - (key2 % 2) * HALF).astype(np.int16)
    hi_[pcs, slot] = il[porder].astype(np.int16)
    slot_of_pair = np.empty(P, np.int64)
    slot_of_pair[porder] = pcs * TS + slot

    wj_t = np.tile(wj.reshape(NCORES, TS // 16, 16).swapaxes(1, 2),
                   (1, 8, 1))
    hi_t = np.tile(hi_.reshape(NCORES, TS // 16, 16).swapaxes(1, 2),
                   (1, 8, 1))

    # h-gather RAW dep block ranges (union over cores per call)
    calls = []
    for off in range(0, LCAP, G):
        calls.append((off, min(G, LCAP - off)))
    for off in range(LCAP, TS, G):
        calls.append((off, min(G, TS - off)))
    hdep_ranges = []
    hblk = hi_ >> 7  # (NCORES, TS)
    for off, sl in calls:
        seg = hblk[:, off:off + sl]
        hdep_ranges.append((int(seg.min()), int(seg.max())))
    hdep_ranges = tuple(hdep_ranges)

    # ---------------- build inputs ----------------
    v_bf = np.ascontiguousarray(v.astype(BF16))
    if use_b:
        w_aug = np.zeros((N, WW), np.float32)
        w_aug[:, :D] = w_np
        w_aug[:, D] = b_np
        w_bf = np.ascontiguousarray(w_aug.astype(BF16))
    else:
        w_bf = np.ascontiguousarray(w_np.astype(BF16))
    mu_bf = np.ascontiguousarray(mu_np.astype(BF16))

    key = (CAPL, CAPH, LCAP, HCAP, WW, hdep_ranges)
    if key not in _BUILD_CACHE:
        _BUILD_CACHE.clear()
        _BUILD_CACHE[key] = _build_program(CAPL, CAPH, LCAP, HCAP, WW,
                                           hdep_ranges)
    nc = _BUILD_CACHE[key]

    in_maps = []
    for c in range(NCORES):
        in_maps.append({
            "v_bf": v_bf,
            "a_t": np.ascontiguousarray(
                A[c].transpose(0, 1, 2, 3).reshape(BLOCKS, 128, CAP * 128)),
            "gi_lo": gi_lo[c],
            "gi_hi": gi_hi[c],
            "mu_bf": mu_bf,
            "w_bf": w_bf,
            "wj_t": wj_t[c],
            "hi_t": hi_t[c],
        })

    res = run_bass_kernel_spmd(
        nc, in_maps, core_ids=list(range(NCORES)),
        trace=bool(int(os.environ.get("KERNEL_TRACE", "0"))))
    LAST_RESULTS = res

    if os.environ.get("KERNEL_BENCH", "0") == "1":
        _benchmark(nc, in_maps)

    flat = np.concatenate(
        [res.results[c]["scores"].T.reshape(-1) for c in range(NCORES)])
    return flat[slot_of_pair].astype(np.float32)


def _benchmark(nc, in_maps, iters=10):
    """Time pure device execution: inputs pre-placed on device, repeated
    jit executions (mirrors bass2jax.run_bass_via_pjrt's multi-core path)."""
    import time

    import jax
    from jax.sharding import Mesh, NamedSharding, PartitionSpec

    from concourse import bass2jax
    from concourse.bass2jax import _bass_exec_p, install_neuronx_cc_hook

    install_neuronx_cc_hook()
    n_cores = NCORES
    part_name = (nc.partition_id_tensor.name
                 if nc.partition_id_tensor else None)
    in_names = []
    out_names = []
    out_avals = []
    zero_outs = []
    for alloc in nc.m.functions[0].allocations:
        if not isinstance(alloc, mybir.MemoryLocationSet):
            continue
        name = alloc.memorylocations[0].name
        if alloc.kind == "ExternalInput":
            if name != part_name:
                in_names.append(name)
        elif alloc.kind == "ExternalOutput":
            out_names.append(name)
            shape = tuple(alloc.tensor_shape)
            dtype = mybir.dt.np(alloc.dtype)
            out_avals.append(jax.core.ShapedArray(shape, dtype))
            zero_outs.append(np.zeros(shape, dtype))
    n_params = len(in_names)
    n_outs = len(out_avals)
    all_names = in_names + out_names
    if part_name is not None:
        all_names = all_names + [part_name]

    nrep = int(os.environ.get("K_NREP", "1"))

    def _body(*args):
        # args: n_params inputs + nrep * n_outs zero buffers
        ins = list(args[:n_params])
        outs_all = []
        for r in range(nrep):
            operands = ins + list(
                args[n_params + r * n_outs:n_params + (r + 1) * n_outs])
            if part_name is not None:
                operands.append(bass2jax.partition_id_tensor())
            outs = _bass_exec_p.bind(
                *operands,
                out_avals=tuple(out_avals),
                in_names=tuple(all_names),
                out_names=tuple(out_names),
                lowering_input_output_aliases=(),
                sim_require_finite=True,
                sim_require_nnan=True,
                nc=nc,
            )
            outs_all.extend(outs)
        return tuple(outs_all)

    devices = jax.devices()[:n_cores]
    mesh = Mesh(np.asarray(devices), ("core",))
    shard_map = bass2jax.shard_map
    n_zeros = nrep * n_outs
    sharded = jax.jit(
        shard_map(_body, mesh=mesh,
                  in_specs=(PartitionSpec("core"),) * (n_params + n_zeros),
                  out_specs=(PartitionSpec("core"),) * n_zeros,
                  check_rep=False),
        donate_argnums=tuple(range(n_params, n_params + n_zeros)),
        keep_unused=True)

    sh = NamedSharding(mesh, PartitionSpec("core"))
    dev_in = [
        jax.device_put(
            np.concatenate([np.asarray(in_maps[c][nm]) for c in
                            range(n_cores)], axis=0), sh)
        for nm in in_names]
    concat_zeros = [np.zeros((n_cores * z.shape[0], *z.shape[1:]), z.dtype)
                    for z in zero_outs] * nrep

    for _ in range(2):
        outs = sharded(*dev_in, *concat_zeros)
        jax.block_until_ready(outs)
    times = []
    for _ in range(iters):
        t0 = time.perf_counter()
        outs = sharded(*dev_in, *concat_zeros)
        jax.block_until_ready(outs)
        times.append(time.perf_counter() - t0)
    times = np.array(times)
    print(f"exec wall: min {times.min()*1e6:.0f} us  "
          f"median {np.median(times)*1e6:.0f} us  "
          f"mean {times.mean()*1e6:.0f} us")
    print(f"HW exec time: {times.min()*1e9:.0f} ns")
